# revision 19
# baseline (speedup 1.0000x reference)
"""Sharded GQA attention (causal + packed-segment mask) for 8 Trainium2 NeuronCores.

Strategy
--------
* Core c handles batch b = c//4 and KV heads {2*(c%4), 2*(c%4)+1} (8 query
  heads per core); the sequence dim stays unsharded.
* decoder_segment_ids are sorted, so the segment mask makes attention
  block-diagonal over contiguous segment spans.  The host reads the actual
  ids, splits each batch into runs, and the device kernel does causal-only
  attention per segment.  The two batches' run structures are unioned
  (padded) so all 8 cores execute one SPMD program; padded "ghost" rows
  contribute nothing (zero K columns give exp(0)=1 but the matching V rows
  and their ones-column are zero, so numerator and denominator are
  unaffected), and ghost query columns produce garbage the host discards.
* Dtypes are chosen per engine roofline: Q/K/mask/ident are fp16 (PE runs
  fp16 at 1 cycle/row vs 4 for fp32; fp16's 10-bit mantissa matches the
  TF32-style rounding fp32r applies anyway), P=exp(S) is written as bf16
  by ScalarE (needs bf16 range: logits are unnormalized, exp can reach
  ~1e27) and V is bf16 to match, with an appended ones column so the
  softmax denominator falls out of the same PV matmuls.  PSUM accumulation
  is fp32 throughout.
* t-slabs are unpadded (the last slab of each segment is L%128 wide),
  cutting exp columns and QK matmul rows ~12%.  Only diagonal chunks need
  masking (pure causal, one shared [128,512] additive tile applied via an
  identity-stationary matmul).  The kernel skips the softmax division
  entirely: it stores the bf16 numerator and denominator (the ones-column
  output) and the host divides during reassembly.
* The steady state is latency-bound (cross-engine semaphore chains), not
  engine-busy-bound, so pipeline depth wins: single-bank S^T slabs 4-deep
  (GRP=1, psum_s bufs=4), double-buffered PV accumulators, and a deep P
  tile pool.  For_i ends every trip with an all-engine barrier, so the
  timed build unrolls several kernel copies per trip; input tiles are
  double-buffered so copy u+1's DMAs overlap copy u's compute.
"""

import math

import numpy as np
import ml_dtypes

B, T, NQ, NKV, D = 2, 1024, 32, 8, 128
G = NQ // NKV
NCORES = 8
KV_PER_CORE = NKV // (NCORES // B)
CHUNK = 128
NEG = -30000.0  # fp16-safe; exp(NEG + max_logit) == 0 in fp32

F16 = np.float16
BF16 = ml_dtypes.bfloat16

_PROGRAM_CACHE = {}


# --------------------------------------------------------------------------
# host-side structure
# --------------------------------------------------------------------------

def _runs(seg_row):
    d = np.flatnonzero(np.diff(seg_row) != 0)
    starts = np.concatenate(([0], d + 1))
    ends = np.concatenate((d + 1, [len(seg_row)]))
    return [(int(s), int(e - s)) for s, e in zip(starts, ends)]


def _structure(ids):
    runs = [_runs(np.asarray(ids[b])) for b in range(B)]
    n_seg = max(len(r) for r in runs)
    L = [max((r[i][1] for r in runs if len(r) > i), default=0) for i in range(n_seg)]
    K = [math.ceil(l / CHUNK) for l in L]
    segs = [i for i in range(n_seg) if K[i] > 0]
    # unpadded t-slab widths
    W = {i: [min(CHUNK, L[i] - j * CHUNK) for j in range(K[i])] for i in segs}
    slabs = [(i, kv_i, j) for i in segs for kv_i in range(KV_PER_CORE)
             for j in range(K[i])]
    chunks = [(i, kv_i, c) for i in segs for kv_i in range(KV_PER_CORE)
              for c in range(K[i])]
    # qT column offset of each slab's [G, w] block
    qoff = {}
    off = 0
    for (i, kv_i, j) in slabs:
        qoff[(i, kv_i, j)] = off
        off += G * W[i][j]
    return runs, L, K, W, segs, slabs, chunks, qoff, off


def _prepare_core(core, q, k, v, runs, L, K, W, segs, slabs, chunks, qoff,
                  qcols):
    b = core // (NCORES // B)
    kv_heads = [KV_PER_CORE * (core % (NCORES // B)) + x for x in range(KV_PER_CORE)]
    rb = runs[b]

    def seg_info(i):
        if i < len(rb):
            return rb[i]
        return (0, 0)

    qT = np.zeros((D, qcols), F16)
    for (i, kv_i, j) in slabs:
        a, lb = seg_info(i)
        t0 = j * CHUNK
        w = W[i][j]
        n_real = min(w, lb - t0)
        if n_real > 0:
            qo = qoff[(i, kv_i, j)]
            for g in range(G):
                h = G * kv_heads[kv_i] + g
                blk = q[b, a + t0:a + t0 + n_real, h, :]  # [n_real, D]
                qT[:, qo + g * w: qo + g * w + n_real] = blk.T.astype(F16)

    kT = np.zeros((D, len(chunks) * CHUNK), F16)
    vO = np.zeros((CHUNK, len(chunks) * 130), BF16)
    for ci, (i, kv_i, c) in enumerate(chunks):
        a, lb = seg_info(i)
        s0 = c * CHUNK
        n_real = min(CHUNK, lb - s0)
        if n_real > 0:
            kvh = kv_heads[kv_i]
            kT[:, ci * CHUNK: ci * CHUNK + n_real] = \
                k[b, a + s0:a + s0 + n_real, kvh, :].T.astype(F16)
            vO[:n_real, ci * 130: ci * 130 + D] = \
                v[b, a + s0:a + s0 + n_real, kvh, :].astype(BF16)
            vO[:n_real, ci * 130 + D] = BF16(1.0)

    # one shared causal mask tile: NEG strictly below the diagonal
    # (s > t), tiled across the G query heads
    sr = np.arange(CHUNK)
    m = np.where(sr[:, None] > sr[None, :], F16(NEG), F16(0.0))
    m4 = np.tile(m, (1, G))

    return {"qT": qT, "kT": kT, "vO": vO, "m4": m4,
            "ident": np.eye(CHUNK, dtype=F16)}


def _assemble(outs, runs, L, K, W, slabs):
    full = np.zeros((B, T, NQ, D), np.float32)
    for core in range(NCORES):
        b = core // (NCORES // B)
        kv_heads = [KV_PER_CORE * (core % (NCORES // B)) + x
                    for x in range(KV_PER_CORE)]
        res = outs[core]  # [NSLAB, 128, 4*130] bf16 numerator||denominator
        rb = runs[b]
        for si, (i, kv_i, j) in enumerate(slabs):
            if i >= len(rb):
                continue
            a, lb = rb[i]
            t0 = j * CHUNK
            n_real = min(W[i][j], lb - t0)
            if n_real <= 0:
                continue
            r = np.asarray(res[si], np.float32).reshape(CHUNK, G, 130)
            for g in range(G):
                h = G * kv_heads[kv_i] + g
                numer = r[:n_real, g, :D]
                denom = r[:n_real, g, D:D + 1]
                full[b, a + t0:a + t0 + n_real, h, :] = numer / denom
    return full


# --------------------------------------------------------------------------
# numpy emulation of the device schedule (debug/validation only)
# --------------------------------------------------------------------------

def _numpy_schedule(ins, L, K, W, segs, slabs, chunks, qoff):
    slab_idx = {s: i for i, s in enumerate(slabs)}
    chunk_idx = {c: i for i, c in enumerate(chunks)}
    qT = ins["qT"].astype(np.float32)
    kT = ins["kT"].astype(np.float32)
    vO = ins["vO"].astype(np.float32)
    m4 = ins["m4"].astype(np.float32)
    out = np.zeros((len(slabs), CHUNK, G * 130), np.float32)
    for i in segs:
        for kv_i in range(KV_PER_CORE):
            for j in range(K[i]):
                w = W[i][j]
                si = slab_idx[(i, kv_i, j)]
                qo = qoff[(i, kv_i, j)]
                ot = np.zeros((CHUNK, G, 130), np.float32)
                for c in range(j + 1):
                    ci = chunk_idx[(i, kv_i, c)]
                    lhsT = kT[:, ci * CHUNK:(ci + 1) * CHUNK]       # [d, s]
                    rhs = qT[:, qo:qo + G * w]                      # [d, (g,w)]
                    S = lhsT.T @ rhs                                # [s, (g,w)]
                    if c == j:
                        mm = m4.reshape(CHUNK, G, CHUNK)[:, :, :w] \
                            .reshape(CHUNK, G * w)
                        S = S + mm
                    P = np.exp(S).astype(BF16).astype(np.float32)
                    vo = vO[:, ci * 130:(ci + 1) * 130]             # [s, 130]
                    for g in range(G):
                        ot[:w, g, :] += P[:, g * w:(g + 1) * w].T @ vo
                out[si] = ot.astype(BF16).astype(np.float32).reshape(CHUNK, G * 130)
    return out.astype(BF16)


# --------------------------------------------------------------------------
# bass program
# --------------------------------------------------------------------------

def _build_program(L, K, W, segs, slabs, chunks, qoff, qcols, loop_n=0,
                   unroll=16):
    import contextlib

    import concourse.bacc as bacc
    import concourse.bass as bass
    import concourse.tile as tile
    from concourse import mybir

    if loop_n:
        assert loop_n % unroll == 0, (loop_n, unroll)
    else:
        unroll = 1

    slab_idx = {s: i for i, s in enumerate(slabs)}
    chunk_idx = {c: i for i, c in enumerate(chunks)}
    f32 = mybir.dt.float32
    f16 = mybir.dt.float16
    bf16 = mybir.dt.bfloat16

    nc = bacc.Bacc()
    qT_d = nc.dram_tensor("qT", [D, qcols], f16, kind="ExternalInput")
    kT_d = nc.dram_tensor("kT", [D, len(chunks) * CHUNK], f16, kind="ExternalInput")
    vO_d = nc.dram_tensor("vO", [CHUNK, len(chunks) * 130], bf16,
                          kind="ExternalInput")
    m4_d = nc.dram_tensor("m4", [CHUNK, 512], f16, kind="ExternalInput")
    id_d = nc.dram_tensor("ident", [CHUNK, CHUNK], f16, kind="ExternalInput")
    out_d = nc.dram_tensor("out", [len(slabs), CHUNK, G * 130], bf16,
                           kind="ExternalOutput")

    with tile.TileContext(nc) as tc:
        with tc.tile_pool(name="pin", bufs=2) as pin, \
             tc.tile_pool(name="pp", bufs=12) as pp, \
             tc.tile_pool(name="po", bufs=3) as po, \
             tc.tile_pool(name="psum_s", bufs=4, space="PSUM") as psum_s, \
             tc.tile_pool(name="psum_o", bufs=2, space="PSUM") as psum_o, \
             (tc.For_i(0, loop_n // unroll, 1) if loop_n else
              contextlib.nullcontext()):

          # For_i ends every trip with an all-engine barrier, so nothing
          # overlaps across trips: a trip costs DMA-lead-in + compute +
          # drain + barrier, serially.  Unrolling puts `unroll` copies of
          # the kernel in one trip so the scheduler overlaps copy u+1's
          # input DMAs (double-buffered via pin bufs=2) with copy u's
          # compute, and the fixed lead-in/drain/barrier amortizes.
          for u in range(unroll):
            ident_t = pin.tile([CHUNK, CHUNK], f16, tag="ident")
            nc.sync.dma_start(out=ident_t[:], in_=id_d[:])
            m4_t = pin.tile([CHUNK, 512], f16, tag="m4")
            nc.sync.dma_start(out=m4_t[:], in_=m4_d[:])

            # inputs, emitted in compute-consumption order so the first
            # segment's tiles land first and compute starts early
            kT_t = {}
            vO_t = {}
            qT_t = {}
            for i in segs:
                for kv_i in range(KV_PER_CORE):
                    ci0 = chunk_idx[(i, kv_i, 0)]
                    kk = K[i]
                    kt = pin.tile([D, kk * CHUNK], f16, tag=f"kT_{i}_{kv_i}")
                    nc.sync.dma_start(out=kt[:],
                                      in_=kT_d[:, ci0 * CHUNK:(ci0 + kk) * CHUNK])
                    kT_t[(i, kv_i)] = kt
                    vt = pin.tile([CHUNK, kk * 130], bf16, tag=f"vO_{i}_{kv_i}")
                    nc.sync.dma_start(out=vt[:],
                                      in_=vO_d[:, ci0 * 130:(ci0 + kk) * 130])
                    vO_t[(i, kv_i)] = vt
                    qo0 = qoff[(i, kv_i, 0)]
                    ncols = sum(G * W[i][j] for j in range(kk))
                    qt = pin.tile([D, ncols], f16, tag=f"qT_{i}_{kv_i}")
                    nc.sync.dma_start(out=qt[:], in_=qT_d[:, qo0:qo0 + ncols])
                    for j in range(kk):
                        o = qoff[(i, kv_i, j)] - qo0
                        qT_t[(i, kv_i, j)] = qt[:, o:o + G * W[i][j]]

            GRP = 1  # s-chunks per PSUM slab (4 single-bank slabs in flight)
            for i in segs:
                for kv_i in range(KV_PER_CORE):
                    kt = kT_t[(i, kv_i)]
                    vt = vO_t[(i, kv_i)]
                    kk = K[i]
                    ostage = po.tile([CHUNK, kk * G * 130], bf16, tag="ostage")
                    for j in range(kk):
                        w = W[i][j]
                        qt = qT_t[(i, kv_i, j)]
                        ot = [psum_o.tile([CHUNK, 2, 132], f32, tag=f"ot{h}",
                                          name=f"ot{h}")
                              for h in range(2)]
                        # diagonal chunk FIRST: its chain (mask matmul + QK
                        # + exp) is the longest, so hoisting it off the slab
                        # tail shortens the critical path; the mask matmul is
                        # emitted before the QK inside the accumulation group
                        # (order within a PSUM group is free) since it only
                        # needs constants and can run arbitrarily early
                        cs = [j] + list(range(j))
                        pt_of = {}
                        for g0 in range(0, j + 1, GRP):
                            grp = cs[g0:g0 + GRP]
                            slab = psum_s.tile([CHUNK, GRP, 512], f32,
                                               tag="slab")
                            for gi, c in enumerate(grp):
                                lhsT = kt[:, c * CHUNK:(c + 1) * CHUNK]
                                if c == j:
                                    mm = m4_t[:].rearrange(
                                        "p (g t) -> p g t", g=G)[:, :, :w]
                                    nc.tensor.matmul(
                                        slab[:, gi, :G * w].rearrange(
                                            "p (g t) -> p g t", g=G),
                                        ident_t[:], mm,
                                        start=True, stop=False)
                                nc.tensor.matmul(
                                    slab[:, gi, :G * w], lhsT, qt,
                                    start=c != j, stop=True)
                            pt = pp.tile([CHUNK, GRP, 512], bf16, tag="pt")
                            nc.scalar.activation(
                                out=pt[:, :len(grp), :G * w],
                                in_=slab[:, :len(grp), :G * w],
                                func=mybir.ActivationFunctionType.Exp)
                            for gi, c in enumerate(grp):
                                pt_of[c] = pt[:, gi, :]
                        for ci, c in enumerate(cs):
                            psl = pt_of[c]
                            vsl = vt[:, c * 130:(c + 1) * 130]
                            for g in range(G):
                                # each ot bank holds two heads but forms ONE
                                # accumulation group: start clears has_written
                                # bank-wide, so only the first matmul into the
                                # bank starts and only the last one stops
                                nc.tensor.matmul(
                                    ot[g // 2][:w, g % 2, 0:130],
                                    psl[:, g * w:(g + 1) * w],
                                    vsl,
                                    start=(ci == 0 and g % 2 == 0),
                                    stop=(ci == j and g % 2 == 1))
                        # drain PSUM -> bf16 SBUF unnormalized (numerator and
                        # ones-column denominator); the host does the divide
                        osl = ostage[:, j * G * 130:(j + 1) * G * 130] \
                            .rearrange("p (u h c) -> p u h c", u=2, h=2)
                        for h in range(2):
                            nc.vector.tensor_copy(
                                out=osl[:, h, :, :], in_=ot[h][:, :, 0:130])
                    si0 = slab_idx[(i, kv_i, 0)]
                    nc.sync.dma_start(
                        out=out_d[si0:si0 + kk].rearrange("k p c -> p k c"),
                        in_=ostage[:].rearrange("p (k c) -> p k c", k=kk))

    nc.finalize()
    return nc


# --------------------------------------------------------------------------
# entry point
# --------------------------------------------------------------------------

def kernel(query, key, value, decoder_segment_ids, _trace=False, _numpy=False):
    query = np.asarray(query, np.float32)
    key = np.asarray(key, np.float32)
    value = np.asarray(value, np.float32)
    ids = np.asarray(decoder_segment_ids)
    # the block-diagonal decomposition relies on segment ids being sorted
    # (contiguous segments), as setup_inputs guarantees
    assert np.all(np.diff(ids.astype(np.int64), axis=-1) >= 0)

    runs, L, K, W, segs, slabs, chunks, qoff, qcols = _structure(ids)
    core_ins = [_prepare_core(c, query, key, value, runs, L, K, W,
                              segs, slabs, chunks, qoff, qcols)
                for c in range(NCORES)]

    if _numpy:
        outs = [_numpy_schedule(ci, L, K, W, segs, slabs, chunks, qoff)
                for ci in core_ins]
        return _assemble(outs, runs, L, K, W, slabs)

    from concourse.bass_utils import run_bass_kernel_spmd

    cache_key = tuple(L)
    if cache_key not in _PROGRAM_CACHE:
        _PROGRAM_CACHE[cache_key] = _build_program(L, K, W, segs, slabs,
                                                   chunks, qoff, qcols)
    nc = _PROGRAM_CACHE[cache_key]

    in_maps = [{k_: v_ for k_, v_ in ci.items()} for ci in core_ins]
    res = run_bass_kernel_spmd(nc, in_maps, list(range(NCORES)), trace=_trace)
    outs = [res.results[c]["out"] for c in range(NCORES)]
    full = _assemble(outs, runs, L, K, W, slabs)
    if _trace:
        return full, res
    return full


# revision 21
# speedup vs baseline: 1.0479x; 1.0479x over previous
"""Sharded GQA attention (causal + packed-segment mask) for 8 Trainium2 NeuronCores.

Strategy
--------
* Core c handles batch b = c//4 and KV heads {2*(c%4), 2*(c%4)+1} (8 query
  heads per core); the sequence dim stays unsharded.
* decoder_segment_ids are sorted, so the segment mask makes attention
  block-diagonal over contiguous segment spans.  The host reads the actual
  ids, splits each batch into runs, and the device kernel does causal-only
  attention per segment.  The two batches' run structures are unioned
  (padded) so all 8 cores execute one SPMD program; padded "ghost" rows
  contribute nothing (zero K columns give exp(0)=1 but the matching V rows
  and their ones-column are zero, so numerator and denominator are
  unaffected), and ghost query columns produce garbage the host discards.
* Dtypes are chosen per engine roofline: Q/K/mask/ident are fp16 (PE runs
  fp16 at 1 cycle/row vs 4 for fp32; fp16's 10-bit mantissa matches the
  TF32-style rounding fp32r applies anyway), P=exp(S) is written as bf16
  by ScalarE (needs bf16 range: logits are unnormalized, exp can reach
  ~1e27) and V is bf16 to match, with an appended ones column so the
  softmax denominator falls out of the same PV matmuls.  PSUM accumulation
  is fp32 throughout.
* t-slabs are unpadded (the last slab of each segment is L%128 wide),
  cutting exp columns and QK matmul rows ~12%.  Only diagonal chunks need
  masking (pure causal, one shared [128,512] additive tile applied via an
  identity-stationary matmul).  The kernel skips the softmax division
  entirely: it stores the bf16 numerator and denominator (the ones-column
  output) and the host divides during reassembly.
* The steady state is latency-bound (cross-engine semaphore chains), not
  engine-busy-bound, so pipeline depth wins: single-bank S^T slabs 4-deep
  (GRP=1, psum_s bufs=4), double-buffered PV accumulators, and a deep P
  tile pool.  For_i ends every trip with an all-engine barrier, so the
  timed build unrolls several kernel copies per trip; input tiles are
  double-buffered so copy u+1's DMAs overlap copy u's compute.
"""

import math

import numpy as np
import ml_dtypes

B, T, NQ, NKV, D = 2, 1024, 32, 8, 128
G = NQ // NKV
NCORES = 8
KV_PER_CORE = NKV // (NCORES // B)
CHUNK = 128
NEG = -30000.0  # fp16-safe; exp(NEG + max_logit) == 0 in fp32

F16 = np.float16
BF16 = ml_dtypes.bfloat16

_PROGRAM_CACHE = {}


# --------------------------------------------------------------------------
# host-side structure
# --------------------------------------------------------------------------

def _runs(seg_row):
    d = np.flatnonzero(np.diff(seg_row) != 0)
    starts = np.concatenate(([0], d + 1))
    ends = np.concatenate((d + 1, [len(seg_row)]))
    return [(int(s), int(e - s)) for s, e in zip(starts, ends)]


def _structure(ids):
    runs = [_runs(np.asarray(ids[b])) for b in range(B)]
    n_seg = max(len(r) for r in runs)
    L = [max((r[i][1] for r in runs if len(r) > i), default=0) for i in range(n_seg)]
    K = [math.ceil(l / CHUNK) for l in L]
    segs = [i for i in range(n_seg) if K[i] > 0]
    # unpadded t-slab widths
    W = {i: [min(CHUNK, L[i] - j * CHUNK) for j in range(K[i])] for i in segs}
    slabs = [(i, kv_i, j) for i in segs for kv_i in range(KV_PER_CORE)
             for j in range(K[i])]
    chunks = [(i, kv_i, c) for i in segs for kv_i in range(KV_PER_CORE)
              for c in range(K[i])]
    # qT column offset of each slab's [G, w] block
    qoff = {}
    off = 0
    for (i, kv_i, j) in slabs:
        qoff[(i, kv_i, j)] = off
        off += G * W[i][j]
    return runs, L, K, W, segs, slabs, chunks, qoff, off


def _prepare_core(core, q, k, v, runs, L, K, W, segs, slabs, chunks, qoff,
                  qcols):
    b = core // (NCORES // B)
    kv_heads = [KV_PER_CORE * (core % (NCORES // B)) + x for x in range(KV_PER_CORE)]
    rb = runs[b]

    def seg_info(i):
        if i < len(rb):
            return rb[i]
        return (0, 0)

    qT = np.zeros((D, qcols), F16)
    for (i, kv_i, j) in slabs:
        a, lb = seg_info(i)
        t0 = j * CHUNK
        w = W[i][j]
        n_real = min(w, lb - t0)
        if n_real > 0:
            qo = qoff[(i, kv_i, j)]
            for g in range(G):
                h = G * kv_heads[kv_i] + g
                blk = q[b, a + t0:a + t0 + n_real, h, :]  # [n_real, D]
                qT[:, qo + g * w: qo + g * w + n_real] = blk.T.astype(F16)

    kT = np.zeros((D, len(chunks) * CHUNK), F16)
    vO = np.zeros((CHUNK, len(chunks) * 130), BF16)
    for ci, (i, kv_i, c) in enumerate(chunks):
        a, lb = seg_info(i)
        s0 = c * CHUNK
        n_real = min(CHUNK, lb - s0)
        if n_real > 0:
            kvh = kv_heads[kv_i]
            kT[:, ci * CHUNK: ci * CHUNK + n_real] = \
                k[b, a + s0:a + s0 + n_real, kvh, :].T.astype(F16)
            vO[:n_real, ci * 130: ci * 130 + D] = \
                v[b, a + s0:a + s0 + n_real, kvh, :].astype(BF16)
            vO[:n_real, ci * 130 + D] = BF16(1.0)

    # one shared causal mask tile: NEG strictly below the diagonal
    # (s > t), tiled across the G query heads
    sr = np.arange(CHUNK)
    m = np.where(sr[:, None] > sr[None, :], F16(NEG), F16(0.0))
    m4 = np.tile(m, (1, G))

    return {"qT": qT, "kT": kT, "vO": vO, "m4": m4,
            "ident": np.eye(CHUNK, dtype=F16)}


def _assemble(outs, runs, L, K, W, slabs):
    full = np.zeros((B, T, NQ, D), np.float32)
    for core in range(NCORES):
        b = core // (NCORES // B)
        kv_heads = [KV_PER_CORE * (core % (NCORES // B)) + x
                    for x in range(KV_PER_CORE)]
        res = outs[core]  # [NSLAB, 128, 4*130] bf16 numerator||denominator
        rb = runs[b]
        for si, (i, kv_i, j) in enumerate(slabs):
            if i >= len(rb):
                continue
            a, lb = rb[i]
            t0 = j * CHUNK
            n_real = min(W[i][j], lb - t0)
            if n_real <= 0:
                continue
            r = np.asarray(res[si], np.float32).reshape(CHUNK, G, 130)
            for g in range(G):
                h = G * kv_heads[kv_i] + g
                numer = r[:n_real, g, :D]
                denom = r[:n_real, g, D:D + 1]
                full[b, a + t0:a + t0 + n_real, h, :] = numer / denom
    return full


# --------------------------------------------------------------------------
# numpy emulation of the device schedule (debug/validation only)
# --------------------------------------------------------------------------

def _numpy_schedule(ins, L, K, W, segs, slabs, chunks, qoff):
    slab_idx = {s: i for i, s in enumerate(slabs)}
    chunk_idx = {c: i for i, c in enumerate(chunks)}
    qT = ins["qT"].astype(np.float32)
    kT = ins["kT"].astype(np.float32)
    vO = ins["vO"].astype(np.float32)
    m4 = ins["m4"].astype(np.float32)
    out = np.zeros((len(slabs), CHUNK, G * 130), np.float32)
    for i in segs:
        for kv_i in range(KV_PER_CORE):
            for j in range(K[i]):
                w = W[i][j]
                si = slab_idx[(i, kv_i, j)]
                qo = qoff[(i, kv_i, j)]
                ot = np.zeros((CHUNK, G, 130), np.float32)
                for c in range(j + 1):
                    ci = chunk_idx[(i, kv_i, c)]
                    lhsT = kT[:, ci * CHUNK:(ci + 1) * CHUNK]       # [d, s]
                    rhs = qT[:, qo:qo + G * w]                      # [d, (g,w)]
                    S = lhsT.T @ rhs                                # [s, (g,w)]
                    if c == j:
                        mm = m4.reshape(CHUNK, G, CHUNK)[:, :, :w] \
                            .reshape(CHUNK, G * w)
                        S = S + mm
                    P = np.exp(S).astype(BF16).astype(np.float32)
                    vo = vO[:, ci * 130:(ci + 1) * 130]             # [s, 130]
                    for g in range(G):
                        ot[:w, g, :] += P[:, g * w:(g + 1) * w].T @ vo
                out[si] = ot.astype(BF16).astype(np.float32).reshape(CHUNK, G * 130)
    return out.astype(BF16)


# --------------------------------------------------------------------------
# bass program
# --------------------------------------------------------------------------

def _build_program(L, K, W, segs, slabs, chunks, qoff, qcols, loop_n=0,
                   unroll=16):
    import contextlib

    import concourse.bacc as bacc
    import concourse.bass as bass
    import concourse.tile as tile
    from concourse import mybir

    if loop_n:
        assert loop_n % unroll == 0, (loop_n, unroll)
    else:
        unroll = 1

    slab_idx = {s: i for i, s in enumerate(slabs)}
    chunk_idx = {c: i for i, c in enumerate(chunks)}
    f32 = mybir.dt.float32
    f16 = mybir.dt.float16
    bf16 = mybir.dt.bfloat16

    nc = bacc.Bacc()
    qT_d = nc.dram_tensor("qT", [D, qcols], f16, kind="ExternalInput")
    kT_d = nc.dram_tensor("kT", [D, len(chunks) * CHUNK], f16, kind="ExternalInput")
    vO_d = nc.dram_tensor("vO", [CHUNK, len(chunks) * 130], bf16,
                          kind="ExternalInput")
    m4_d = nc.dram_tensor("m4", [CHUNK, 512], f16, kind="ExternalInput")
    id_d = nc.dram_tensor("ident", [CHUNK, CHUNK], f16, kind="ExternalInput")
    out_d = nc.dram_tensor("out", [len(slabs), CHUNK, G * 130], bf16,
                           kind="ExternalOutput")

    with tile.TileContext(nc) as tc:
        with tc.tile_pool(name="pin", bufs=3) as pin, \
             tc.tile_pool(name="pp", bufs=12) as pp, \
             tc.tile_pool(name="po", bufs=3) as po, \
             tc.tile_pool(name="psum_s", bufs=4, space="PSUM") as psum_s, \
             tc.tile_pool(name="psum_o", bufs=2, space="PSUM") as psum_o, \
             (tc.For_i(0, loop_n // unroll, 1) if loop_n else
              contextlib.nullcontext()):

          # For_i ends every trip with an all-engine barrier, so nothing
          # overlaps across trips: a trip costs DMA-lead-in + compute +
          # drain + barrier, serially.  Unrolling puts `unroll` copies of
          # the kernel in one trip so the scheduler overlaps copy u+1's
          # input DMAs (double-buffered via pin bufs=2) with copy u's
          # compute, and the fixed lead-in/drain/barrier amortizes.
          for u in range(unroll):
            ident_t = pin.tile([CHUNK, CHUNK], f16, tag="ident")
            nc.sync.dma_start(out=ident_t[:], in_=id_d[:])
            m4_t = pin.tile([CHUNK, 512], f16, tag="m4")
            nc.sync.dma_start(out=m4_t[:], in_=m4_d[:])

            # inputs, emitted in compute-consumption order so the first
            # segment's tiles land first and compute starts early
            kT_t = {}
            vO_t = {}
            qT_t = {}
            for i in segs:
                for kv_i in range(KV_PER_CORE):
                    ci0 = chunk_idx[(i, kv_i, 0)]
                    kk = K[i]
                    kt = pin.tile([D, kk * CHUNK], f16, tag=f"kT_{i}_{kv_i}")
                    nc.sync.dma_start(out=kt[:],
                                      in_=kT_d[:, ci0 * CHUNK:(ci0 + kk) * CHUNK])
                    kT_t[(i, kv_i)] = kt
                    vt = pin.tile([CHUNK, kk * 130], bf16, tag=f"vO_{i}_{kv_i}")
                    nc.sync.dma_start(out=vt[:],
                                      in_=vO_d[:, ci0 * 130:(ci0 + kk) * 130])
                    vO_t[(i, kv_i)] = vt
                    qo0 = qoff[(i, kv_i, 0)]
                    ncols = sum(G * W[i][j] for j in range(kk))
                    qt = pin.tile([D, ncols], f16, tag=f"qT_{i}_{kv_i}")
                    nc.sync.dma_start(out=qt[:], in_=qT_d[:, qo0:qo0 + ncols])
                    for j in range(kk):
                        o = qoff[(i, kv_i, j)] - qo0
                        qT_t[(i, kv_i, j)] = qt[:, o:o + G * W[i][j]]

            GRP = 1  # s-chunks per PSUM slab (4 single-bank slabs in flight)
            for i in segs:
                for kv_i in range(KV_PER_CORE):
                    kt = kT_t[(i, kv_i)]
                    vt = vO_t[(i, kv_i)]
                    kk = K[i]
                    ostage = po.tile([CHUNK, kk * G * 130], bf16, tag="ostage")
                    for j in range(kk):
                        w = W[i][j]
                        qt = qT_t[(i, kv_i, j)]
                        ot = [psum_o.tile([CHUNK, 2, 132], f32, tag=f"ot{h}",
                                          name=f"ot{h}")
                              for h in range(2)]
                        pts = []
                        for g0 in range(0, j + 1, GRP):
                            grp = list(range(g0, min(g0 + GRP, j + 1)))
                            slab = psum_s.tile([CHUNK, GRP, 512], f32,
                                               tag="slab")
                            for gi, c in enumerate(grp):
                                lhsT = kt[:, c * CHUNK:(c + 1) * CHUNK]
                                masked = c == j
                                nc.tensor.matmul(
                                    slab[:, gi, :G * w], lhsT, qt,
                                    start=True, stop=not masked)
                                if masked:
                                    mm = m4_t[:].rearrange(
                                        "p (g t) -> p g t", g=G)[:, :, :w]
                                    nc.tensor.matmul(
                                        slab[:, gi, :G * w].rearrange(
                                            "p (g t) -> p g t", g=G),
                                        ident_t[:], mm,
                                        start=False, stop=True)
                            pt = pp.tile([CHUNK, GRP, 512], bf16, tag="pt")
                            nc.scalar.activation(
                                out=pt[:, :len(grp), :G * w],
                                in_=slab[:, :len(grp), :G * w],
                                func=mybir.ActivationFunctionType.Exp)
                            pts.append(pt)
                        for c in range(j + 1):
                            psl = pts[c // GRP][:, c % GRP, :]
                            vsl = vt[:, c * 130:(c + 1) * 130]
                            for g in range(G):
                                # each ot bank holds two heads but forms ONE
                                # accumulation group: start clears has_written
                                # bank-wide, so only the first matmul into the
                                # bank starts and only the last one stops
                                nc.tensor.matmul(
                                    ot[g // 2][:w, g % 2, 0:130],
                                    psl[:, g * w:(g + 1) * w],
                                    vsl,
                                    start=(c == 0 and g % 2 == 0),
                                    stop=(c == j and g % 2 == 1))
                        # drain PSUM -> bf16 SBUF unnormalized (numerator and
                        # ones-column denominator); the host does the divide
                        osl = ostage[:, j * G * 130:(j + 1) * G * 130] \
                            .rearrange("p (u h c) -> p u h c", u=2, h=2)
                        for h in range(2):
                            nc.vector.tensor_copy(
                                out=osl[:, h, :, :], in_=ot[h][:, :, 0:130])
                    si0 = slab_idx[(i, kv_i, 0)]
                    nc.sync.dma_start(
                        out=out_d[si0:si0 + kk].rearrange("k p c -> p k c"),
                        in_=ostage[:].rearrange("p (k c) -> p k c", k=kk))

    nc.finalize()
    return nc


# --------------------------------------------------------------------------
# entry point
# --------------------------------------------------------------------------

def kernel(query, key, value, decoder_segment_ids, _trace=False, _numpy=False):
    query = np.asarray(query, np.float32)
    key = np.asarray(key, np.float32)
    value = np.asarray(value, np.float32)
    ids = np.asarray(decoder_segment_ids)
    # the block-diagonal decomposition relies on segment ids being sorted
    # (contiguous segments), as setup_inputs guarantees
    assert np.all(np.diff(ids.astype(np.int64), axis=-1) >= 0)

    runs, L, K, W, segs, slabs, chunks, qoff, qcols = _structure(ids)
    core_ins = [_prepare_core(c, query, key, value, runs, L, K, W,
                              segs, slabs, chunks, qoff, qcols)
                for c in range(NCORES)]

    if _numpy:
        outs = [_numpy_schedule(ci, L, K, W, segs, slabs, chunks, qoff)
                for ci in core_ins]
        return _assemble(outs, runs, L, K, W, slabs)

    from concourse.bass_utils import run_bass_kernel_spmd

    cache_key = tuple(L)
    if cache_key not in _PROGRAM_CACHE:
        _PROGRAM_CACHE[cache_key] = _build_program(L, K, W, segs, slabs,
                                                   chunks, qoff, qcols)
    nc = _PROGRAM_CACHE[cache_key]

    in_maps = [{k_: v_ for k_, v_ in ci.items()} for ci in core_ins]
    res = run_bass_kernel_spmd(nc, in_maps, list(range(NCORES)), trace=_trace)
    outs = [res.results[c]["out"] for c in range(NCORES)]
    full = _assemble(outs, runs, L, K, W, slabs)
    if _trace:
        return full, res
    return full


# revision 24
# speedup vs baseline: 1.6182x; 1.5442x over previous
"""Sharded GQA attention (causal + packed-segment mask) for 8 Trainium2 NeuronCores.

Strategy
--------
* Core c handles batch b = c//4 and KV heads {2*(c%4), 2*(c%4)+1} (8 query
  heads per core); the sequence dim stays unsharded.
* decoder_segment_ids are sorted, so the segment mask makes attention
  block-diagonal over contiguous segment spans.  The host reads the actual
  ids, splits each batch into runs, and the device kernel does causal-only
  attention per segment.  The two batches' run structures are unioned
  (padded) so all 8 cores execute one SPMD program; padded "ghost" rows
  contribute nothing (zero K columns give exp(0)=1 but the matching V rows
  and their ones-column are zero, so numerator and denominator are
  unaffected), and ghost query columns produce garbage the host discards.
* Dtypes are chosen per engine roofline: Q/K/mask/ident are fp16 (PE runs
  fp16 at 1 cycle/row vs 4 for fp32; fp16's 10-bit mantissa matches the
  TF32-style rounding fp32r applies anyway), P=exp(S) is written as bf16
  by ScalarE (needs bf16 range: logits are unnormalized, exp can reach
  ~1e27) and V is bf16 to match, with an appended ones column so the
  softmax denominator falls out of the same PV matmuls.  PSUM accumulation
  is fp32 throughout.
* t-slabs are unpadded (the last slab of each segment is L%128 wide),
  cutting exp columns and QK matmul rows ~12%.  Only diagonal chunks need
  masking (pure causal, one shared [128,512] additive tile applied via an
  identity-stationary matmul).  The kernel skips the softmax division
  entirely: it stores the bf16 numerator and denominator (the ones-column
  output) and the host divides during reassembly.
* The steady state is latency-bound (cross-engine semaphore chains), not
  engine-busy-bound, so pipeline depth wins: single-bank S^T slabs 4-deep
  (GRP=1, psum_s bufs=4), double-buffered PV accumulators, and a deep P
  tile pool.  For_i ends every trip with an all-engine barrier, so the
  timed build unrolls several kernel copies per trip; input tiles are
  double-buffered so copy u+1's DMAs overlap copy u's compute.
"""

import math

import numpy as np
import ml_dtypes

B, T, NQ, NKV, D = 2, 1024, 32, 8, 128
G = NQ // NKV
NCORES = 8
KV_PER_CORE = NKV // (NCORES // B)
CHUNK = 128
NEG = -30000.0  # fp16-safe; exp(NEG + max_logit) == 0 in fp32

F16 = np.float16
BF16 = ml_dtypes.bfloat16

_PROGRAM_CACHE = {}


# --------------------------------------------------------------------------
# host-side structure
# --------------------------------------------------------------------------

def _runs(seg_row):
    d = np.flatnonzero(np.diff(seg_row) != 0)
    starts = np.concatenate(([0], d + 1))
    ends = np.concatenate((d + 1, [len(seg_row)]))
    return [(int(s), int(e - s)) for s, e in zip(starts, ends)]


def _structure(ids):
    runs = [_runs(np.asarray(ids[b])) for b in range(B)]
    n_seg = max(len(r) for r in runs)
    L = [max((r[i][1] for r in runs if len(r) > i), default=0) for i in range(n_seg)]
    K = [math.ceil(l / CHUNK) for l in L]
    segs = [i for i in range(n_seg) if K[i] > 0]
    # unpadded t-slab widths
    W = {i: [min(CHUNK, L[i] - j * CHUNK) for j in range(K[i])] for i in segs}
    slabs = [(i, kv_i, j) for i in segs for kv_i in range(KV_PER_CORE)
             for j in range(K[i])]
    chunks = [(i, kv_i, c) for i in segs for kv_i in range(KV_PER_CORE)
              for c in range(K[i])]
    # qT column offset of each slab's [G, w] block
    qoff = {}
    off = 0
    for (i, kv_i, j) in slabs:
        qoff[(i, kv_i, j)] = off
        off += G * W[i][j]
    return runs, L, K, W, segs, slabs, chunks, qoff, off


def _prepare_core(core, q, k, v, runs, L, K, W, segs, slabs, chunks, qoff,
                  qcols):
    b = core // (NCORES // B)
    kv_heads = [KV_PER_CORE * (core % (NCORES // B)) + x for x in range(KV_PER_CORE)]
    rb = runs[b]

    def seg_info(i):
        if i < len(rb):
            return rb[i]
        return (0, 0)

    qT = np.zeros((D, qcols), F16)
    for (i, kv_i, j) in slabs:
        a, lb = seg_info(i)
        t0 = j * CHUNK
        w = W[i][j]
        n_real = min(w, lb - t0)
        if n_real > 0:
            qo = qoff[(i, kv_i, j)]
            for g in range(G):
                h = G * kv_heads[kv_i] + g
                blk = q[b, a + t0:a + t0 + n_real, h, :]  # [n_real, D]
                qT[:, qo + g * w: qo + g * w + n_real] = blk.T.astype(F16)

    kT = np.zeros((D, len(chunks) * CHUNK), F16)
    vO = np.zeros((CHUNK, len(chunks) * 130), BF16)
    for ci, (i, kv_i, c) in enumerate(chunks):
        a, lb = seg_info(i)
        s0 = c * CHUNK
        n_real = min(CHUNK, lb - s0)
        if n_real > 0:
            kvh = kv_heads[kv_i]
            kT[:, ci * CHUNK: ci * CHUNK + n_real] = \
                k[b, a + s0:a + s0 + n_real, kvh, :].T.astype(F16)
            vO[:n_real, ci * 130: ci * 130 + D] = \
                v[b, a + s0:a + s0 + n_real, kvh, :].astype(BF16)
            vO[:n_real, ci * 130 + D] = BF16(1.0)

    # one shared causal mask tile: NEG strictly below the diagonal
    # (s > t), tiled across the G query heads
    sr = np.arange(CHUNK)
    m = np.where(sr[:, None] > sr[None, :], F16(NEG), F16(0.0))
    m4 = np.tile(m, (1, G))

    return {"qT": qT, "kT": kT, "vO": vO, "m4": m4,
            "ident": np.eye(CHUNK, dtype=F16)}


def _assemble(outs, runs, L, K, W, slabs):
    full = np.zeros((B, T, NQ, D), np.float32)
    for core in range(NCORES):
        b = core // (NCORES // B)
        kv_heads = [KV_PER_CORE * (core % (NCORES // B)) + x
                    for x in range(KV_PER_CORE)]
        res = outs[core]  # [NSLAB, 128, 4*130] bf16 numerator||denominator
        rb = runs[b]
        for si, (i, kv_i, j) in enumerate(slabs):
            if i >= len(rb):
                continue
            a, lb = rb[i]
            t0 = j * CHUNK
            n_real = min(W[i][j], lb - t0)
            if n_real <= 0:
                continue
            r = np.asarray(res[si], np.float32).reshape(CHUNK, G, 130)
            for g in range(G):
                h = G * kv_heads[kv_i] + g
                numer = r[:n_real, g, :D]
                denom = r[:n_real, g, D:D + 1]
                full[b, a + t0:a + t0 + n_real, h, :] = numer / denom
    return full


# --------------------------------------------------------------------------
# numpy emulation of the device schedule (debug/validation only)
# --------------------------------------------------------------------------

def _numpy_schedule(ins, L, K, W, segs, slabs, chunks, qoff):
    slab_idx = {s: i for i, s in enumerate(slabs)}
    chunk_idx = {c: i for i, c in enumerate(chunks)}
    qT = ins["qT"].astype(np.float32)
    kT = ins["kT"].astype(np.float32)
    vO = ins["vO"].astype(np.float32)
    m4 = ins["m4"].astype(np.float32)
    out = np.zeros((len(slabs), CHUNK, G * 130), np.float32)
    for i in segs:
        for kv_i in range(KV_PER_CORE):
            for j in range(K[i]):
                w = W[i][j]
                si = slab_idx[(i, kv_i, j)]
                qo = qoff[(i, kv_i, j)]
                ot = np.zeros((CHUNK, G, 130), np.float32)
                for c in range(j + 1):
                    ci = chunk_idx[(i, kv_i, c)]
                    lhsT = kT[:, ci * CHUNK:(ci + 1) * CHUNK]       # [d, s]
                    rhs = qT[:, qo:qo + G * w]                      # [d, (g,w)]
                    S = lhsT.T @ rhs                                # [s, (g,w)]
                    if c == j:
                        mm = m4.reshape(CHUNK, G, CHUNK)[:, :, :w] \
                            .reshape(CHUNK, G * w)
                        S = S + mm
                    P = np.exp(S).astype(BF16).astype(np.float32)
                    vo = vO[:, ci * 130:(ci + 1) * 130]             # [s, 130]
                    for g in range(G):
                        ot[:w, g, :] += P[:, g * w:(g + 1) * w].T @ vo
                out[si] = ot.astype(BF16).astype(np.float32).reshape(CHUNK, G * 130)
    return out.astype(BF16)


# --------------------------------------------------------------------------
# bass program
# --------------------------------------------------------------------------

def _build_program(L, K, W, segs, slabs, chunks, qoff, qcols, loop_n=0,
                   unroll=16):
    import contextlib

    import concourse.bacc as bacc
    import concourse.bass as bass
    import concourse.tile as tile
    from concourse import mybir

    if loop_n:
        assert loop_n % unroll == 0, (loop_n, unroll)
    else:
        unroll = 1

    slab_idx = {s: i for i, s in enumerate(slabs)}
    chunk_idx = {c: i for i, c in enumerate(chunks)}
    f32 = mybir.dt.float32
    f16 = mybir.dt.float16
    bf16 = mybir.dt.bfloat16

    nc = bacc.Bacc()
    qT_d = nc.dram_tensor("qT", [D, qcols], f16, kind="ExternalInput")
    kT_d = nc.dram_tensor("kT", [D, len(chunks) * CHUNK], f16, kind="ExternalInput")
    vO_d = nc.dram_tensor("vO", [CHUNK, len(chunks) * 130], bf16,
                          kind="ExternalInput")
    m4_d = nc.dram_tensor("m4", [CHUNK, 512], f16, kind="ExternalInput")
    id_d = nc.dram_tensor("ident", [CHUNK, CHUNK], f16, kind="ExternalInput")
    out_d = nc.dram_tensor("out", [len(slabs), CHUNK, G * 130], bf16,
                           kind="ExternalOutput")

    with tile.TileContext(nc) as tc:
        with tc.tile_pool(name="pin", bufs=2) as pin, \
             tc.tile_pool(name="pp", bufs=12) as pp, \
             tc.tile_pool(name="po", bufs=4) as po, \
             tc.tile_pool(name="psum_s", bufs=4, space="PSUM") as psum_s, \
             tc.tile_pool(name="psum_o", bufs=2, space="PSUM") as psum_o, \
             (tc.For_i(0, loop_n // unroll, 1) if loop_n else
              contextlib.nullcontext()):

          # For_i ends every trip with an all-engine barrier, so nothing
          # overlaps across trips: a trip costs DMA-lead-in + compute +
          # drain + barrier, serially.  Unrolling puts `unroll` copies of
          # the kernel in one trip so the scheduler overlaps copy u+1's
          # input DMAs (double-buffered via pin bufs=2) with copy u's
          # compute, and the fixed lead-in/drain/barrier amortizes.
          for u in range(unroll):
            ident_t = pin.tile([CHUNK, CHUNK], f16, tag="ident")
            nc.sync.dma_start(out=ident_t[:], in_=id_d[:])
            m4_t = pin.tile([CHUNK, 512], f16, tag="m4")
            nc.sync.dma_start(out=m4_t[:], in_=m4_d[:])

            # inputs, emitted in compute-consumption order so the first
            # segment's tiles land first and compute starts early
            kT_t = {}
            vO_t = {}
            qT_t = {}
            for i in segs:
                for kv_i in range(KV_PER_CORE):
                    ci0 = chunk_idx[(i, kv_i, 0)]
                    kk = K[i]
                    kt = pin.tile([D, kk * CHUNK], f16, tag=f"kT_{i}_{kv_i}")
                    nc.sync.dma_start(out=kt[:],
                                      in_=kT_d[:, ci0 * CHUNK:(ci0 + kk) * CHUNK])
                    kT_t[(i, kv_i)] = kt
                    vt = pin.tile([CHUNK, kk * 130], bf16, tag=f"vO_{i}_{kv_i}")
                    nc.sync.dma_start(out=vt[:],
                                      in_=vO_d[:, ci0 * 130:(ci0 + kk) * 130])
                    vO_t[(i, kv_i)] = vt
                    qo0 = qoff[(i, kv_i, 0)]
                    ncols = sum(G * W[i][j] for j in range(kk))
                    qt = pin.tile([D, ncols], f16, tag=f"qT_{i}_{kv_i}")
                    nc.sync.dma_start(out=qt[:], in_=qT_d[:, qo0:qo0 + ncols])
                    for j in range(kk):
                        o = qoff[(i, kv_i, j)] - qo0
                        qT_t[(i, kv_i, j)] = qt[:, o:o + G * W[i][j]]

            GRP = 1  # s-chunks per PSUM slab (4 single-bank slabs in flight)
            for i in segs:
                for kv_i in range(KV_PER_CORE):
                    kt = kT_t[(i, kv_i)]
                    vt = vO_t[(i, kv_i)]
                    kk = K[i]
                    ostage = po.tile([CHUNK, kk * G * 130], bf16, tag="ostage")
                    for j in range(kk):
                        w = W[i][j]
                        qt = qT_t[(i, kv_i, j)]
                        ot = [psum_o.tile([CHUNK, 2, 132], f32, tag=f"ot{h}",
                                          name=f"ot{h}")
                              for h in range(2)]
                        pts = []
                        for g0 in range(0, j + 1, GRP):
                            grp = list(range(g0, min(g0 + GRP, j + 1)))
                            slab = psum_s.tile([CHUNK, GRP, 512], f32,
                                               tag="slab")
                            for gi, c in enumerate(grp):
                                lhsT = kt[:, c * CHUNK:(c + 1) * CHUNK]
                                masked = c == j
                                nc.tensor.matmul(
                                    slab[:, gi, :G * w], lhsT, qt,
                                    start=True, stop=not masked)
                                if masked:
                                    mm = m4_t[:].rearrange(
                                        "p (g t) -> p g t", g=G)[:, :, :w]
                                    nc.tensor.matmul(
                                        slab[:, gi, :G * w].rearrange(
                                            "p (g t) -> p g t", g=G),
                                        ident_t[:], mm,
                                        start=False, stop=True)
                            pt = pp.tile([CHUNK, GRP, 512], bf16, tag="pt")
                            nc.scalar.activation(
                                out=pt[:, :len(grp), :G * w],
                                in_=slab[:, :len(grp), :G * w],
                                func=mybir.ActivationFunctionType.Exp)
                            pts.append(pt)
                        for c in range(j + 1):
                            psl = pts[c // GRP][:, c % GRP, :]
                            vsl = vt[:, c * 130:(c + 1) * 130]
                            for g in range(G):
                                # each ot bank holds two heads but forms ONE
                                # accumulation group: start clears has_written
                                # bank-wide, so only the first matmul into the
                                # bank starts and only the last one stops
                                nc.tensor.matmul(
                                    ot[g // 2][:w, g % 2, 0:130],
                                    psl[:, g * w:(g + 1) * w],
                                    vsl,
                                    start=(c == 0 and g % 2 == 0),
                                    stop=(c == j and g % 2 == 1))
                        # drain PSUM -> bf16 SBUF unnormalized (numerator and
                        # ones-column denominator); the host does the divide
                        osl = ostage[:, j * G * 130:(j + 1) * G * 130] \
                            .rearrange("p (u h c) -> p u h c", u=2, h=2)
                        for h in range(2):
                            nc.vector.tensor_copy(
                                out=osl[:, h, :, :], in_=ot[h][:, :, 0:130])
                    si0 = slab_idx[(i, kv_i, 0)]
                    # stores ride the GPSIMD SWDGE queue: the SP HWDGE queue
                    # is ~96% busy with input DMAs + stores in the scheduled
                    # steady state, so stores get their own (idle) pipe
                    nc.gpsimd.dma_start(
                        out=out_d[si0:si0 + kk].rearrange("k p c -> p k c"),
                        in_=ostage[:].rearrange("p (k c) -> p k c", k=kk))

    nc.finalize()
    return nc


# --------------------------------------------------------------------------
# entry point
# --------------------------------------------------------------------------

def kernel(query, key, value, decoder_segment_ids, _trace=False, _numpy=False):
    query = np.asarray(query, np.float32)
    key = np.asarray(key, np.float32)
    value = np.asarray(value, np.float32)
    ids = np.asarray(decoder_segment_ids)
    # the block-diagonal decomposition relies on segment ids being sorted
    # (contiguous segments), as setup_inputs guarantees
    assert np.all(np.diff(ids.astype(np.int64), axis=-1) >= 0)

    runs, L, K, W, segs, slabs, chunks, qoff, qcols = _structure(ids)
    core_ins = [_prepare_core(c, query, key, value, runs, L, K, W,
                              segs, slabs, chunks, qoff, qcols)
                for c in range(NCORES)]

    if _numpy:
        outs = [_numpy_schedule(ci, L, K, W, segs, slabs, chunks, qoff)
                for ci in core_ins]
        return _assemble(outs, runs, L, K, W, slabs)

    from concourse.bass_utils import run_bass_kernel_spmd

    cache_key = tuple(L)
    if cache_key not in _PROGRAM_CACHE:
        _PROGRAM_CACHE[cache_key] = _build_program(L, K, W, segs, slabs,
                                                   chunks, qoff, qcols)
    nc = _PROGRAM_CACHE[cache_key]

    in_maps = [{k_: v_ for k_, v_ in ci.items()} for ci in core_ins]
    res = run_bass_kernel_spmd(nc, in_maps, list(range(NCORES)), trace=_trace)
    outs = [res.results[c]["out"] for c in range(NCORES)]
    full = _assemble(outs, runs, L, K, W, slabs)
    if _trace:
        return full, res
    return full
